# revision 6
# baseline (speedup 1.0000x reference)
"""Multi-scale deformable attention (nearest sampling, sum over points) on
8 Trainium2 NeuronCores via Bass/Tile.

Sharding: the 240000 (batch*query) rows split into 24 phases of 10000
queries (4 phases per batch); each core runs 3 phases (30000 rows), so no
phase straddles a batch boundary.

Gather strategy: instead of the GpSimd ap_gather ucode (~110 ns per index,
serial on the pool engine), each sample row is fetched from HBM by the
SWDGE dma_gather instruction: the Q7 pair generates one 256-B descriptor
per (query, head, level, point) event (~0.34 ns/descriptor) and the DMA
engines execute them. The value table is laid out per head-pair
[4][14960][64] f32 so a 256-B row covers one head pair at one key and the
row index fits int16. Sampling x/y come in pre-transposed from the host so
index arithmetic is 9 vector ops per block; the gather writes queries to
partitions and (level,point) to free-dim slots, so the point-sum is a
single strided tensor_reduce and the output tile needs no transposes.

No cross-core communication; inputs/outputs are sharded/assembled on host.
"""
import numpy as np

SPATIAL = [(64, 176), (32, 88), (16, 44), (8, 22)]
LVL_OFF = [0, 11264, 14080, 14784]
NKEY = 14960
BS, NQ = 6, 40000
QPP, PHASES, QB = 10000, 3, 128
NBLK = 79            # ceil(10000/128); last block overlaps (q0 = 9872)
N_CORES = 8
MAGIC = 12582912.0   # 1.5 * 2**23 : float32 round-to-nearest-even bias
NCOL = 256           # idx-stream columns per block: 32 slots * 8 qm-groups

_CACHE = {}


def _make_consts():
    """[128, 5*NCOL] f32: W, H, XC2, XHI, YHI tiles, column layout
    col = slot*8 + qmh, slot = (l*4+p)*2 + h2."""
    c = np.zeros((128, 5 * NCOL), np.float32)
    for col in range(NCOL):
        slot = col // 8
        l = (slot // 2) // 4
        h_l, w_l = SPATIAL[l]
        off = LVL_OFF[l]
        c[:, 0 * NCOL + col] = w_l
        c[:, 1 * NCOL + col] = h_l
        c[:, 2 * NCOL + col] = MAGIC - off
        c[:, 3 * NCOL + col] = off + w_l - 1
        c[:, 4 * NCOL + col] = MAGIC + h_l - 1
    return c


def _build_program():
    from concourse import bacc, tile, mybir, library_config

    F32 = mybir.dt.float32
    I16 = mybir.dt.int16
    A = mybir.AluOpType

    nc = bacc.Bacc("TRN2", target_bir_lowering=False, debug=False)
    # value rows per (phase, headpair): 256-B rows, one per key.
    values4 = nc.dram_tensor("values4", [PHASES, 4, NKEY, 64], F32,
                             kind="ExternalInput")
    # pre-transposed sampling planes: [ph, blk, part, (x|y), col]
    sampxy = nc.dram_tensor("sampxy", [PHASES, NBLK, 128, 2, NCOL], F32,
                            kind="ExternalInput")
    consts = nc.dram_tensor("consts", [128, 5 * NCOL], F32,
                            kind="ExternalInput")
    out_ext = nc.dram_tensor("out", [PHASES * QPP, 256], F32,
                             kind="ExternalOutput")

    with tile.TileContext(nc) as tc:
        with tc.tile_pool(name="cst", bufs=1) as cstp, \
             tc.tile_pool(name="xy", bufs=3) as xyp, \
             tc.tile_pool(name="tf", bufs=2) as tfp, \
             tc.tile_pool(name="tix", bufs=2) as tixp, \
             tc.tile_pool(name="tall", bufs=2) as tallp, \
             tc.tile_pool(name="g", bufs=2) as gp, \
             tc.tile_pool(name="r", bufs=2) as rp, \
             tc.tile_pool(name="ost", bufs=3) as ostp:

            cst = cstp.tile([128, 5 * NCOL], F32, tag="cst")
            nc.sync.dma_start(out=cst[:], in_=consts[:])
            Wt = cst[:, 0 * NCOL:1 * NCOL]
            Ht = cst[:, 1 * NCOL:2 * NCOL]
            XCt = cst[:, 2 * NCOL:3 * NCOL]
            XHIt = cst[:, 3 * NCOL:4 * NCOL]
            YHIt = cst[:, 4 * NCOL:5 * NCOL]

            nc.gpsimd.load_library(library_config.mlp)

            for ph in range(PHASES):
                for blk in range(NBLK):
                    q0 = min(blk * QB, QPP - QB)
                    xy = xyp.tile([128, 2, NCOL], F32, tag="xy")
                    nc.sync.dma_start(out=xy[:], in_=sampxy[ph, blk])
                    xs = xy[:, 0, :]
                    ys = xy[:, 1, :]
                    # ix' = min(MAGIC + rint(x*w - .5) - (MAGIC - off),
                    #           off + w - 1)
                    tx = tfp.tile([128, NCOL], F32, tag="tx")
                    nc.vector.tensor_tensor(out=tx[:], in0=xs, in1=Wt,
                                            op=A.mult)
                    nc.vector.tensor_scalar(out=tx[:], in0=tx[:],
                                            scalar1=-0.5, scalar2=MAGIC,
                                            op0=A.add, op1=A.add)
                    nc.vector.tensor_tensor(out=tx[:], in0=tx[:], in1=XCt,
                                            op=A.subtract)
                    nc.vector.tensor_tensor(out=tx[:], in0=tx[:], in1=XHIt,
                                            op=A.min)
                    # uy = min(MAGIC + rint(y*h - .5), MAGIC + h - 1)
                    ty = tfp.tile([128, NCOL], F32, tag="ty")
                    nc.vector.tensor_tensor(out=ty[:], in0=ys, in1=Ht,
                                            op=A.mult)
                    nc.vector.tensor_scalar(out=ty[:], in0=ty[:],
                                            scalar1=-0.5, scalar2=MAGIC,
                                            op0=A.add, op1=A.add)
                    nc.vector.tensor_tensor(out=ty[:], in0=ty[:], in1=YHIt,
                                            op=A.min)
                    # idx = (uy - MAGIC)*w + ix'  (exact ints in f32 -> i16)
                    nc.vector.scalar_tensor_tensor(
                        out=ty[:], in0=ty[:], scalar=MAGIC, in1=Wt,
                        op0=A.subtract, op1=A.mult)
                    tix = tixp.tile([128, NCOL], I16, tag="tix")
                    nc.vector.tensor_tensor(out=tix[:], in0=ty[:], in1=tx[:],
                                            op=A.add)
                    # stage the four head-pair streams at partitions 0..32
                    tall = tallp.tile([128, 4, NCOL], I16, tag="tall")
                    for hp in range(4):
                        nc.sync.dma_start(
                            out=tall[0:32, hp, :],
                            in_=tix[32 * hp:32 * hp + 32, :])
                    gs = []
                    for hp in range(4):
                        g = gp.tile([128, 32, 64], F32, tag=f"g{hp}")
                        # SWDGE ring fits ~1K descriptors per call: split the
                        # 4096-event stream into 4 ring-sized calls.
                        for qt in range(4):
                            nc.gpsimd.dma_gather(
                                out_ap=g[:, 8 * qt:8 * qt + 8, :],
                                in_ap=values4[ph, hp],
                                idxs_ap=tall[:, hp, 64 * qt:64 * qt + 64],
                                num_idxs=8 * QB,
                                num_idxs_reg=8 * QB,
                                elem_size=64,
                            )
                        r = rp.tile([128, 2, 64], F32, tag=f"r{hp}")
                        nc.vector.tensor_reduce(
                            out=r[:],
                            in_=g[:].rearrange("p (l h) c -> p h c l", h=2),
                            axis=mybir.AxisListType.X, op=A.add)
                        gs.append(r)
                    ost = ostp.tile([128, 256], F32, tag="ost")
                    for hp in range(4):
                        for h2 in range(2):
                            c0 = (2 * hp + h2) * 32
                            nc.scalar.copy(
                                out=ost[:, c0:c0 + 32],
                                in_=gs[hp][:, h2, h2 * 32:h2 * 32 + 32])
                    row0 = ph * QPP + q0
                    nc.sync.dma_start(out=out_ext[row0:row0 + QB, :],
                                      in_=ost[:])
    return nc


def _compile_spmd(nc, n_cores):
    """Compile-once runner based on concourse.bass2jax.run_bass_via_pjrt."""
    import jax
    from jax.sharding import Mesh, PartitionSpec, NamedSharding
    try:
        from jax.experimental.shard_map import shard_map
    except ImportError:
        from jax.shard_map import shard_map
    from concourse import mybir
    from concourse.bass2jax import (
        install_neuronx_cc_hook, _bass_exec_p, partition_id_tensor)

    install_neuronx_cc_hook()
    if not nc.is_finalized():
        nc.finalize()
    partition_name = (nc.partition_id_tensor.name
                      if nc.partition_id_tensor else None)

    in_names, out_names, out_avals, zero_outs = [], [], [], []
    for alloc in nc.m.functions[0].allocations:
        if not isinstance(alloc, mybir.MemoryLocationSet):
            continue
        name = alloc.memorylocations[0].name
        if alloc.kind == "ExternalInput":
            if name != partition_name:
                in_names.append(name)
        elif alloc.kind == "ExternalOutput":
            out_names.append(name)
            shape = tuple(alloc.tensor_shape)
            dtype = mybir.dt.np(alloc.dtype)
            out_avals.append(jax.core.ShapedArray(shape, dtype))
            zero_outs.append(np.zeros(shape, dtype))
    n_params = len(in_names)
    all_in_names = (in_names + out_names
                    + ([partition_name] if partition_name else []))

    def _body(*args):
        operands = list(args)
        if partition_name is not None:
            operands.append(partition_id_tensor())
        outs = _bass_exec_p.bind(
            *operands,
            out_avals=tuple(out_avals),
            in_names=tuple(all_in_names),
            out_names=tuple(out_names),
            lowering_input_output_aliases=(),
            sim_require_finite=True,
            sim_require_nnan=True,
            nc=nc,
        )
        return tuple(outs)

    devices = jax.devices()[:n_cores]
    mesh = Mesh(np.asarray(devices), ("core",))
    in_specs = (PartitionSpec("core"),) * (n_params + len(out_names))
    out_specs = (PartitionSpec("core"),) * len(out_names)
    sharded = jax.jit(
        shard_map(_body, mesh=mesh, in_specs=in_specs,
                  out_specs=out_specs, check_rep=False),
        keep_unused=True,
    )
    sh = NamedSharding(mesh, PartitionSpec("core"))

    def prep(in_maps):
        staged = [
            jax.device_put(
                np.concatenate([m[name] for m in in_maps], axis=0), sh)
            for name in in_names
        ]
        staged += [
            jax.device_put(np.concatenate([z] * n_cores, axis=0), sh)
            for z in zero_outs
        ]
        return staged

    def run(staged):
        return sharded(*staged)
    return run, prep, in_names, out_names


def _get_compiled():
    if "run" not in _CACHE:
        nc = _build_program()
        run, prep, in_names, out_names = _compile_spmd(nc, N_CORES)
        _CACHE.update(run=run, prep=prep, in_names=in_names,
                      out_names=out_names)
    return _CACHE


def _shard_inputs(value, sampling_locations):
    vflat = np.ascontiguousarray(value.reshape(BS, NKEY, 256))
    samp = sampling_locations  # (BS, NQ, 8, 4, 4, 2)
    consts = _make_consts()
    q0s = np.minimum(np.arange(NBLK) * QB, QPP - QB)
    # qloc[blk, qm] -> local query index
    qloc = q0s[:, None] + np.arange(QB)[None, :]
    in_maps = []
    for c in range(N_CORES):
        v4 = np.empty((PHASES, 4, NKEY, 64), np.float32)
        sxy = np.empty((PHASES, NBLK, 128, 2, NCOL), np.float32)
        for ph in range(PHASES):
            row0 = 30000 * c + QPP * ph
            b, qoff = row0 // NQ, row0 % NQ
            for hp in range(4):
                v4[ph, hp] = vflat[b, :, 64 * hp:64 * hp + 64]
            S = samp[b, qoff:qoff + QPP]          # [10000, 8, 4, 4, 2]
            Aq = S[qloc]                          # [blk, 128, 8, 4, 4, 2]
            Aq = Aq.reshape(NBLK, 8, 16, 4, 2, 4, 4, 2)  # qmh r16 hp h2 l p xy
            # -> [blk, hp, r16, l, p, h2, qmh, xy]
            Aq = Aq.transpose(0, 3, 2, 5, 6, 4, 1, 7)
            # duplicate r16 into the two 16-wraps of each 32-partition group
            Aq = np.broadcast_to(Aq[:, :, None], (NBLK, 4, 2, 16, 4, 4, 2, 8, 2))
            # [blk, (hp dup r16)=128, (l p h2 qmh)=256, xy]
            Aq = Aq.reshape(NBLK, 128, NCOL, 2)
            sxy[ph, :, :, 0, :] = Aq[..., 0]
            sxy[ph, :, :, 1, :] = Aq[..., 1]
        in_maps.append({"values4": v4, "sampxy": sxy, "consts": consts})
    return in_maps


def kernel(value, value_spatial_shapes, sampling_locations):
    import jax
    value = np.asarray(value, np.float32)
    sampling_locations = np.asarray(sampling_locations, np.float32)
    cc = _get_compiled()
    in_maps = _shard_inputs(value, sampling_locations)
    staged = cc["prep"](in_maps)
    outs = cc["run"](staged)
    jax.block_until_ready(outs)
    full = np.asarray(outs[0])                 # (8*30000, 256)
    return np.ascontiguousarray(full.reshape(BS, NQ, 256))


# revision 8
# speedup vs baseline: 7.6771x; 7.6771x over previous
"""Multi-scale deformable attention (nearest sampling, sum over points) on
8 Trainium2 NeuronCores via Bass/Tile.

Sharding: the 240000 (batch*query) rows split into 24 phases of 10000
queries (4 phases per batch); each core runs 3 phases (30000 rows), so no
phase straddles a batch boundary.

Gather: GpSimd ap_gather with a head-per-Q7-core d=2 bf16 table.
T2[16k+r, key, c2] holds channel 32k+2r+c2 of head k, so one index step
on core k fetches all 32 channels of head k at one key (16 lanes x d=2)
— halving the serial index stream vs a channel-split d=1 layout (the two
16-partition groups of a 32-channel head no longer duplicate the stream).
The per-core stream for a 256-query block is 4096 indices (16 level-point
slots per query), one ap_gather call per block. bf16 keeps the table at
60KB/partition and ~1e-3 relative error, far under the 2e-2 gate.

Sampling x/y come in host-pre-transposed to the [(head,level,point), q]
plane layout, so index arithmetic is 9 vector ops with per-partition
constants. The point-sum is one strided tensor_reduce; PE transposes with
a strided permuted copy assemble [q, 256] output rows.

No cross-core communication; inputs/outputs are sharded/assembled on host.
"""
import numpy as np

SPATIAL = [(64, 176), (32, 88), (16, 44), (8, 22)]
LVL_OFF = [0, 11264, 14080, 14784]
NKEY = 14960
BS, NQ = 6, 40000
QPP, PHASES, QB = 10000, 3, 256
NBLK = 40            # ceil(10000/256); last block overlaps (q0 = 9744)
N_CORES = 8
MAGIC = 12582912.0   # 1.5 * 2**23 : float32 round-to-nearest-even bias

_CACHE = {}


def _make_consts():
    """[128, 5] f32 per-partition constants; partition p = h*16 + l*4 + pt."""
    c = np.zeros((128, 5), np.float32)
    for p in range(128):
        l = (p % 16) // 4
        h_l, w_l = SPATIAL[l]
        off = LVL_OFF[l]
        c[p, 0] = w_l
        c[p, 1] = h_l
        c[p, 2] = MAGIC - off
        c[p, 3] = off + w_l - 1
        c[p, 4] = MAGIC + h_l - 1
    return c


def _build_program():
    from concourse import bacc, tile, mybir, library_config

    F32 = mybir.dt.float32
    BF16 = mybir.dt.bfloat16
    I16 = mybir.dt.int16
    A = mybir.AluOpType

    nc = bacc.Bacc("TRN2", target_bir_lowering=False, debug=False)
    values2 = nc.dram_tensor("values2", [PHASES, 128, NKEY * 2], BF16,
                             kind="ExternalInput")
    sampxy = nc.dram_tensor("sampxy", [PHASES, NBLK, 128, 2, QB], F32,
                            kind="ExternalInput")
    consts = nc.dram_tensor("consts", [128, 5], F32, kind="ExternalInput")
    ident_in = nc.dram_tensor("ident", [128, 128], F32, kind="ExternalInput")
    out_ext = nc.dram_tensor("out", [PHASES * QPP, 256], F32,
                             kind="ExternalOutput")

    with tile.TileContext(nc) as tc:
        with tc.tile_pool(name="cst", bufs=1) as cstp, \
             tc.tile_pool(name="tab", bufs=1) as tabp, \
             tc.tile_pool(name="xy", bufs=3) as xyp, \
             tc.tile_pool(name="tf", bufs=2) as tfp, \
             tc.tile_pool(name="tix", bufs=2) as tixp, \
             tc.tile_pool(name="g", bufs=2) as gp, \
             tc.tile_pool(name="r", bufs=2) as rp, \
             tc.tile_pool(name="ost", bufs=3) as ostp, \
             tc.tile_pool(name="ps", bufs=4, space="PSUM") as psp:

            cst = cstp.tile([128, 5], F32, tag="cst")
            nc.sync.dma_start(out=cst[:], in_=consts[:])
            idn = cstp.tile([128, 128], F32, tag="idn")
            nc.sync.dma_start(out=idn[:], in_=ident_in[:])
            W = cst[:, 0:1]
            Hh = cst[:, 1:2]
            XC = cst[:, 2:3]
            XHI = cst[:, 3:4]
            YHI2 = cst[:, 4:5]

            nc.gpsimd.load_library(library_config.ap_gather)

            tab = tabp.tile([128, NKEY * 2], BF16, tag="tab")
            for ph in range(PHASES):
                nc.sync.dma_start(out=tab[:], in_=values2[ph])
                for blk in range(NBLK):
                    q0 = min(blk * QB, QPP - QB)
                    xy = xyp.tile([128, 2, QB], F32, tag="xy")
                    nc.sync.dma_start(out=xy[:], in_=sampxy[ph, blk])
                    xs = xy[:, 0, :]
                    ys = xy[:, 1, :]
                    # ix' = min(MAGIC + rint(x*w - .5) - (MAGIC - off),
                    #           off + w - 1)
                    tx = tfp.tile([128, QB], F32, tag="tx")
                    nc.vector.tensor_scalar(out=tx[:], in0=xs,
                                            scalar1=W, scalar2=None,
                                            op0=A.mult)
                    nc.vector.tensor_scalar(out=tx[:], in0=tx[:],
                                            scalar1=-0.5, scalar2=MAGIC,
                                            op0=A.add, op1=A.add)
                    nc.vector.tensor_scalar(out=tx[:], in0=tx[:],
                                            scalar1=XC, scalar2=XHI,
                                            op0=A.subtract, op1=A.min)
                    # uy = min(MAGIC + rint(y*h - .5), MAGIC + h - 1)
                    ty = tfp.tile([128, QB], F32, tag="ty")
                    nc.vector.tensor_scalar(out=ty[:], in0=ys,
                                            scalar1=Hh, scalar2=None,
                                            op0=A.mult)
                    nc.vector.tensor_scalar(out=ty[:], in0=ty[:],
                                            scalar1=-0.5, scalar2=MAGIC,
                                            op0=A.add, op1=A.add)
                    nc.vector.tensor_scalar(out=ty[:], in0=ty[:],
                                            scalar1=YHI2, scalar2=MAGIC,
                                            op0=A.min, op1=A.subtract)
                    # idx = iy*w + ix'  (exact ints in f32 -> i16)
                    tix = tixp.tile([128, QB], I16, tag="tix")
                    nc.vector.scalar_tensor_tensor(
                        out=tix[:], in0=ty[:], scalar=W, in1=tx[:],
                        op0=A.mult, op1=A.add)
                    # one gather per block: core k = head k, d=2 channels
                    g = gp.tile([128, 16 * QB * 2], BF16, tag="g")
                    nc.gpsimd.ap_gather(
                        out_ap=g[:].rearrange("p (j d) -> p j d", d=2),
                        in_ap=tab[:].rearrange("p (k d) -> p k d", d=2),
                        idxs_ap=tix[:], channels=128, num_elems=NKEY,
                        d=2, num_idxs=16 * QB)
                    # sum the 16 level-point slots per query
                    r = rp.tile([128, QB, 2], F32, tag="r")
                    nc.vector.tensor_reduce(
                        out=r[:],
                        in_=g[:].rearrange("p (q l d) -> p q d l", l=16, d=2),
                        axis=mybir.AxisListType.X, op=A.add)
                    # transpose to [q, 256]: ch(p=16a+b, c2) = 32a + 2b + c2
                    for qh in range(2):
                        ost = ostp.tile([128, 256], F32, tag=f"ost{qh}",
                                        name=f"ost{qh}")
                        for c2 in range(2):
                            ps = psp.tile([128, 128], F32, tag=f"ps{c2}",
                                          name=f"ps{c2}")
                            nc.tensor.transpose(
                                ps[:], r[:, 128 * qh:128 * qh + 128, c2],
                                idn[:])
                            src = ps[:].rearrange("q (a b) -> q a b", a=8)
                            dst = ost[:].rearrange(
                                "q (a b t) -> q a b t", a=8, b=16)[:, :, :, c2]
                            nc.scalar.copy(out=dst, in_=src)
                        row0 = ph * QPP + q0 + 128 * qh
                        nc.sync.dma_start(out=out_ext[row0:row0 + 128, :],
                                          in_=ost[:])
    return nc


def _compile_spmd(nc, n_cores):
    """Compile-once runner based on concourse.bass2jax.run_bass_via_pjrt."""
    import jax
    from jax.sharding import Mesh, PartitionSpec, NamedSharding
    try:
        from jax.experimental.shard_map import shard_map
    except ImportError:
        from jax.shard_map import shard_map
    from concourse import mybir
    from concourse.bass2jax import (
        install_neuronx_cc_hook, _bass_exec_p, partition_id_tensor)

    install_neuronx_cc_hook()
    if not nc.is_finalized():
        nc.finalize()
    partition_name = (nc.partition_id_tensor.name
                      if nc.partition_id_tensor else None)

    in_names, out_names, out_avals, zero_outs = [], [], [], []
    for alloc in nc.m.functions[0].allocations:
        if not isinstance(alloc, mybir.MemoryLocationSet):
            continue
        name = alloc.memorylocations[0].name
        if alloc.kind == "ExternalInput":
            if name != partition_name:
                in_names.append(name)
        elif alloc.kind == "ExternalOutput":
            out_names.append(name)
            shape = tuple(alloc.tensor_shape)
            dtype = mybir.dt.np(alloc.dtype)
            out_avals.append(jax.core.ShapedArray(shape, dtype))
            zero_outs.append(np.zeros(shape, dtype))
    n_params = len(in_names)
    all_in_names = (in_names + out_names
                    + ([partition_name] if partition_name else []))

    def _body(*args):
        operands = list(args)
        if partition_name is not None:
            operands.append(partition_id_tensor())
        outs = _bass_exec_p.bind(
            *operands,
            out_avals=tuple(out_avals),
            in_names=tuple(all_in_names),
            out_names=tuple(out_names),
            lowering_input_output_aliases=(),
            sim_require_finite=True,
            sim_require_nnan=True,
            nc=nc,
        )
        return tuple(outs)

    devices = jax.devices()[:n_cores]
    mesh = Mesh(np.asarray(devices), ("core",))
    in_specs = (PartitionSpec("core"),) * (n_params + len(out_names))
    out_specs = (PartitionSpec("core"),) * len(out_names)
    sharded = jax.jit(
        shard_map(_body, mesh=mesh, in_specs=in_specs,
                  out_specs=out_specs, check_rep=False),
        keep_unused=True,
    )
    sh = NamedSharding(mesh, PartitionSpec("core"))

    def prep(in_maps):
        staged = [
            jax.device_put(
                np.concatenate([m[name] for m in in_maps], axis=0), sh)
            for name in in_names
        ]
        staged += [
            jax.device_put(np.concatenate([z] * n_cores, axis=0), sh)
            for z in zero_outs
        ]
        return staged

    def run(staged):
        return sharded(*staged)
    return run, prep, in_names, out_names


def _get_compiled():
    if "run" not in _CACHE:
        nc = _build_program()
        run, prep, in_names, out_names = _compile_spmd(nc, N_CORES)
        _CACHE.update(run=run, prep=prep, in_names=in_names,
                      out_names=out_names)
    return _CACHE


def _shard_inputs(value, sampling_locations):
    import ml_dtypes
    vflat = np.ascontiguousarray(value.reshape(BS, NKEY, 256))
    samp = sampling_locations  # (BS, NQ, 8, 4, 4, 2)
    consts = _make_consts()
    ident = np.eye(128, dtype=np.float32)
    q0s = np.minimum(np.arange(NBLK) * QB, QPP - QB)
    qloc = q0s[:, None] + np.arange(QB)[None, :]      # [blk, q]
    # channel map for the d=2 head-per-core table
    pp = np.arange(128)
    chmap = (32 * (pp[:, None] // 16) + 2 * (pp[:, None] % 16)
             + np.arange(2)[None, :])                 # [128, 2]
    in_maps = []
    for c in range(N_CORES):
        v2 = np.empty((PHASES, 128, NKEY * 2), ml_dtypes.bfloat16)
        sxy = np.empty((PHASES, NBLK, 128, 2, QB), np.float32)
        for ph in range(PHASES):
            row0 = 30000 * c + QPP * ph
            b, qoff = row0 // NQ, row0 % NQ
            # [key, 128, 2] -> [128, key*2]
            tabs = vflat[b][:, chmap].transpose(1, 0, 2)
            v2[ph] = tabs.reshape(128, NKEY * 2).astype(ml_dtypes.bfloat16)
            S = samp[b, qoff:qoff + QPP]              # [10000, 8, 4, 4, 2]
            Aq = S[qloc]                              # [blk, q, 8, 4, 4, 2]
            # -> [blk, (h l p), xy, q]
            Aq = Aq.transpose(0, 2, 3, 4, 5, 1).reshape(NBLK, 128, 2, QB)
            sxy[ph] = Aq
        in_maps.append({"values2": v2, "sampxy": sxy, "consts": consts,
                        "ident": ident})
    return in_maps


def kernel(value, value_spatial_shapes, sampling_locations):
    import jax
    value = np.asarray(value, np.float32)
    sampling_locations = np.asarray(sampling_locations, np.float32)
    cc = _get_compiled()
    in_maps = _shard_inputs(value, sampling_locations)
    staged = cc["prep"](in_maps)
    outs = cc["run"](staged)
    jax.block_until_ready(outs)
    full = np.asarray(outs[0])                 # (8*30000, 256)
    return np.ascontiguousarray(full.reshape(BS, NQ, 256))
